# revision 49
# baseline (speedup 1.0000x reference)
"""Trainium2 Bass kernel for nn_Expert_13082470383822.

y = silu(depthwise_causal_conv1d(x, conv_w, K=4) + conv_b);  out = y @ W_proj.T + b_proj
x [4, 4096, 2048] fp32. Data-parallel over the 16384 (batch*seq) tokens across
8 NeuronCores (2048 tokens/core + 3-token halo).

bf16 datapath. Per-core schedule:
- ~56 dummy N=64 matmuls at kernel start keep the PE busy through the HAM
  activity window so the real matmuls run at 2.4 GHz from the first issue.
- the projection runs j-outer phases of 8 PSUM banks covering ALL 4 m-tiles
  x 2 e-chunks of a 512-token strip; each W row-tile is split into column
  halves (wa = features 0-1023, wb = 1024-2047) so phase A only streams wa
  (half the W bandwidth demand of a full-tile phase) while wb arrives for
  phase B. Strips 0-2 are phase-based; strip 3 is e-outer per m-tile so the
  final copybacks overlap the remaining accumulation.
- each phase processes j14/j15 per bank-PAIR at staggered times so the 8
  PSUM copybacks (split 4 ACT / 4 DVE) pipeline with the next phase's
  restart instead of serializing against it.
- conv per j (full width): tap0 via DVE tensor_scalar, taps 2,3 via
  scalar_tensor_tensor, tap1 on ACT (Copy with per-partition scale), one
  DVE tensor_tensor combine, SiLU+conv_b on ACT. Conv for strip c+1 runs a
  full strip ahead of its phases.
- DMA: x0/x1 ride the ACT HWDGE queue (its triggers fire earliest), W
  halves + x3 + output stores on the sync queue in first-use order, consts
  + x2 on the SWDGE gpsimd queue. b_proj is added on the host.
"""

import sys

if "/opt/trn_rl_repo" not in sys.path:
    sys.path.insert(0, "/opt/trn_rl_repo")

import os

import numpy as np

if os.environ.get("BASS_LDW_OPT", "0") == "1":
    import concourse.bass_utils as _bu

    if not getattr(_bu, "_ldw_opt_patched", False):
        _orig_run_command = _bu.run_command

        def _run_command_ldw(cmd, *a, **kw):
            cmd = [
                "--enable-ldw-opt=true" if c == "--enable-ldw-opt=false" else c
                for c in cmd
            ]
            return _orig_run_command(cmd, *a, **kw)

        _bu.run_command = _run_command_ldw
        _bu._ldw_opt_patched = True

B, S, D, KW = 4, 4096, 2048, 4
NCORES = 8
T = (B * S) // NCORES  # tokens per core = 2048
KT = D // 128  # 16 channel tiles
ECH = D // 512  # 4 e-chunks of the output features
CW = 512  # conv strip width (tokens)
MS = 128  # matmul stationary strip width (tokens)
NCS = T // CW  # 4 conv strips
MPC = CW // MS  # 4 matmul strips per conv strip
JQ = 4  # j-tiles per x quarter-load
NDUM = 88  # warm-up matmuls

_BUILT = {}


def _build_program():
    if "nc" in _BUILT:
        return _BUILT["nc"]

    import concourse.tile as tile
    from concourse import bacc, mybir

    dt = mybir.dt
    AF = mybir.ActivationFunctionType
    ALU = mybir.AluOpType

    nc = bacc.Bacc("TRN2", target_bir_lowering=False, debug=False)
    # pre-tiled x: per (conv-strip, j-quarter): [128, 4*(CW+3)] bf16 contiguous
    xs_d = nc.declare_dram_parameter(
        "xs_t", [NCS * (KT // JQ), 128, JQ * (CW + 3)], dt.bfloat16, isOutput=False
    )
    # tiny j0-only head tiles of strip 0 (tokens -3..257 and 253..517) to
    # start conv early
    xs0_d = nc.declare_dram_parameter("xs0", [128, 260], dt.bfloat16, isOutput=False)
    xs0b_d = nc.declare_dram_parameter("xs0b", [128, 264], dt.bfloat16, isOutput=False)
    wt = nc.declare_dram_parameter("wt", [D, D], dt.bfloat16, isOutput=False)
    cw = nc.declare_dram_parameter("cw", [128, KT * KW], dt.float32, isOutput=False)
    cb = nc.declare_dram_parameter("cb", [128, KT], dt.float32, isOutput=False)
    out = nc.declare_dram_parameter("out", [T, D], dt.bfloat16, isOutput=True)

    with tile.TileContext(nc) as tc:
        with (
            tc.tile_pool(name="consts", bufs=1) as cpool,
            tc.tile_pool(name="wpool", bufs=1) as wpool,
            tc.tile_pool(name="xpool", bufs=12) as xpool,
            tc.tile_pool(name="ypool", bufs=3) as ypool,
            tc.tile_pool(name="apool", bufs=6) as apool,
            tc.tile_pool(name="opool", bufs=12) as opool,
            tc.tile_pool(name="pspool", bufs=8, space="PSUM") as pspool,
        ):
            xq = {}

            def load_xq(c, q, eng):
                xt = xpool.tile([128, JQ, CW + 3], dt.bfloat16, name="xs", tag="xs")
                eng.dma_start(
                    out=xt[:, :, :],
                    in_=xs_d[c * (KT // JQ) + q, :, :].rearrange(
                        "p (j t) -> p j t", j=JQ
                    ),
                )
                xq[(c, q)] = xt

            # ---- startup-critical x tiles ride the ACT HWDGE queue: its
            # triggers fire before the sync queue's preamble finishes; few
            # enough that the ring never blocks ACT compute ----
            xs0 = cpool.tile([128, 260], dt.bfloat16, name="xs0")
            nc.scalar.dma_start(out=xs0[:, :], in_=xs0_d[:, :])
            xs0b = cpool.tile([128, 264], dt.bfloat16, name="xs0b")
            nc.scalar.dma_start(out=xs0b[:, :], in_=xs0b_d[:, :])
            load_xq(0, 0, nc.scalar)
            load_xq(0, 1, nc.scalar)

            # ---- warm-up: ACT table + HAM via dummy matmuls ----
            dmw = cpool.tile([128, 64], dt.bfloat16, name="dmw")
            nc.vector.memset(dmw[:, :], 0.0)
            dum = cpool.tile([1, 1], dt.float32, name="dum")
            nc.vector.memset(dum[:, :], 0.0)
            nc.scalar.activation(dum[:, :], dum[:, :], AF.Silu, bias=0.0)
            dps = pspool.tile([128, 512], dt.float32, name="ps", tag="ps")
            for i in range(NDUM):
                nc.tensor.matmul(
                    dps[0:64, 0:64],
                    dmw[:, 0:64],
                    dmw[:, 0:64],
                    start=(i == 0),
                    stop=(i == NDUM - 1),
                )

            # ---- consts + the rest of x0, x1, x2 on the SWDGE gpsimd queue ----
            cw_sb = cpool.tile([128, KT * KW], dt.float32, name="cw_sb")
            nc.gpsimd.dma_start(out=cw_sb[:, :], in_=cw[:, :])
            cb_sb = cpool.tile([128, KT], dt.float32, name="cb_sb")
            nc.gpsimd.dma_start(out=cb_sb[:, :], in_=cb[:, :])
            load_xq(0, 2, nc.gpsimd)
            load_xq(0, 3, nc.gpsimd)
            for q in range(4):
                load_xq(1, q, nc.gpsimd)
            for q in range(4):
                load_xq(2, q, nc.gpsimd)

            # ---- W column-halves on sync in first-use order, then x3 ----
            wa = [None] * KT
            wb = [None] * KT
            for j in range(0, KT):
                wj = wpool.tile([128, 1024], dt.bfloat16, name=f"wa{j}")
                nc.sync.dma_start(
                    out=wj[:, :], in_=wt[j * 128 : (j + 1) * 128, 0:1024]
                )
                wa[j] = wj
            for j in range(KT):
                wj = wpool.tile([128, 1024], dt.bfloat16, name=f"wb{j}")
                nc.sync.dma_start(
                    out=wj[:, :], in_=wt[j * 128 : (j + 1) * 128, 1024:2048]
                )
                wb[j] = wj
            for q in range(4):
                load_xq(3, q, nc.sync)

            def wsel(j, e):
                w = wa[j] if e < 2 else wb[j]
                return w[:, (e % 2) * 512 : (e % 2) * 512 + 512]

            # ---- conv emission ----
            ys_strip = {}

            def conv_j0_half(h):
                """Strip-0 j0 in halves via pure-DVE chains (lowest latency:
                reads the tiny head tiles that land first)."""
                ys = ys_strip[0]
                tile0 = xs0 if h == 0 else xs0b
                t1 = apool.tile([128, 512], dt.bfloat16, name="t1", tag="t1")
                nc.vector.tensor_scalar(
                    t1[:, 0:256], tile0[:, 0:256], cw_sb[:, 0:1], None, ALU.mult
                )
                for k in range(1, KW):
                    nc.vector.scalar_tensor_tensor(
                        t1[:, 0:256], tile0[:, k : k + 256], cw_sb[:, k : k + 1],
                        t1[:, 0:256], ALU.mult, ALU.add,
                    )
                nc.scalar.activation(
                    ys[:, 0, h * 256 : h * 256 + 256], t1[:, 0:256], AF.Silu,
                    bias=cb_sb[:, 0:1],
                )

            def conv_j(c, j):
                xs = xq[(c, j // JQ)]
                jj = j % JQ
                ys = ys_strip[c]
                t2 = apool.tile([128, 512], dt.bfloat16, name="t2", tag="t2")
                nc.scalar.activation(
                    t2[:, :], xs[:, jj, 1 : 1 + CW], AF.Copy,
                    scale=cw_sb[:, j * KW + 1 : j * KW + 2],
                )
                t1 = apool.tile([128, 512], dt.bfloat16, name="t1", tag="t1")
                nc.vector.tensor_scalar(
                    t1[:, :], xs[:, jj, 0:CW], cw_sb[:, j * KW : j * KW + 1],
                    None, ALU.mult,
                )
                for k in (2, 3):
                    nc.vector.scalar_tensor_tensor(
                        t1[:, :], xs[:, jj, k : k + CW],
                        cw_sb[:, j * KW + k : j * KW + k + 1],
                        t1[:, :], ALU.mult, ALU.add,
                    )
                t3 = apool.tile([128, 512], dt.bfloat16, name="t3", tag="t3")
                nc.vector.tensor_tensor(t3[:, :], t1[:, :], t2[:, :], ALU.add)
                nc.scalar.activation(
                    ys[:, j, :], t3[:, :], AF.Silu, bias=cb_sb[:, j : j + 1]
                )

            # ---- matmul phases: all 4 m-tiles x one e-half, j-outer ----
            def mm_phase(c, eh):
                ys = ys_strip[c]
                pss = [
                    pspool.tile([128, 512], dt.float32, name="ps", tag="ps")
                    for _ in range(8)
                ]

                def mm(b, j):
                    m, el = b // 2, b % 2
                    nc.tensor.matmul(
                        pss[b][:, :],
                        ys[:, j, m * MS : (m + 1) * MS],
                        wsel(j, 2 * eh + el),
                        start=(j == 0),
                        stop=(j == KT - 1),
                    )

                # j0..13 round-robin; last two j per bank-PAIR staggered so
                # copybacks pipeline with the next phase's restarts
                for j in range(KT - 2):
                    for b in range(8):
                        mm(b, j)
                for bp in range(4):
                    for j in (KT - 2, KT - 1):
                        for b in (2 * bp, 2 * bp + 1):
                            mm(b, j)
                return pss

            def copyback(c, eh, pss):
                for b in range(8):
                    m, el = b // 2, b % 2
                    e = 2 * eh + el
                    s = c * MPC + m
                    os_sb = opool.tile([128, 512], dt.bfloat16, name="os", tag="os")
                    if b % 2 == 0:
                        nc.scalar.copy(os_sb[:, :], pss[b][:, :])
                    else:
                        nc.vector.tensor_copy(os_sb[:, :], pss[b][:, :])
                    nc.sync.dma_start(
                        out=out[s * MS : (s + 1) * MS, e * 512 : (e + 1) * 512],
                        in_=os_sb[:, :],
                    )

            # ================= schedule =================
            ys_strip[0] = ypool.tile([128, KT, CW], dt.bfloat16, name="ys", tag="ys")
            conv_j0_half(0)
            conv_j0_half(1)
            for j in range(1, KT):
                conv_j(0, j)

            pss_a = mm_phase(0, 0)

            ys_strip[1] = ypool.tile([128, KT, CW], dt.bfloat16, name="ys", tag="ys")
            conv_j(1, 0)
            copyback(0, 0, pss_a)

            pss_b = mm_phase(0, 1)

            for j in range(1, 11):
                conv_j(1, j)
            copyback(0, 1, pss_b)
            for j in range(11, KT):
                conv_j(1, j)

            pss_c = mm_phase(1, 0)

            ys_strip[2] = ypool.tile([128, KT, CW], dt.bfloat16, name="ys", tag="ys")
            for j in range(0, 7):
                conv_j(2, j)
            copyback(1, 0, pss_c)

            pss_d = mm_phase(1, 1)

            for j in range(7, KT):
                conv_j(2, j)
            ys_strip[3] = ypool.tile([128, KT, CW], dt.bfloat16, name="ys", tag="ys")
            for j in range(0, 5):
                conv_j(3, j)
            copyback(1, 1, pss_d)

            # strips 2+3: conv is a full strip ahead by now, so e-outer per m
            # (copybacks overlap the next accumulation, no phase boundaries);
            # strip-3 conv interleaves with strip-2's m-tiles
            s3_conv = iter(range(5, KT))

            def emit_strip_eouter(c, conv_after_m=0):
                ys = ys_strip[c]
                for m in range(MPC):
                    s = c * MPC + m
                    pss = []
                    for e in range(ECH):
                        ps = pspool.tile([128, 512], dt.float32, name="ps", tag="ps")
                        for j in range(KT):
                            nc.tensor.matmul(
                                ps[:, :],
                                ys[:, j, m * MS : (m + 1) * MS],
                                wsel(j, e),
                                start=(j == 0),
                                stop=(j == KT - 1),
                            )
                        pss.append(ps)
                    for _ in range(conv_after_m):
                        j3 = next(s3_conv, None)
                        if j3 is not None:
                            conv_j(3, j3)
                    last = c == NCS - 1 and m == MPC - 1
                    for e in range(ECH):
                        os_sb = opool.tile(
                            [128, 512], dt.bfloat16, name="os", tag="os"
                        )
                        if last and e >= ECH - 2:
                            # final tiles: split the copy across ACT+DVE to
                            # shave the drain tail
                            nc.scalar.copy(os_sb[:, 0:256], pss[e][:, 0:256])
                            nc.vector.tensor_copy(
                                os_sb[:, 256:512], pss[e][:, 256:512]
                            )
                        elif e % 2 == 0:
                            nc.scalar.copy(os_sb[:, :], pss[e][:, :])
                        else:
                            nc.vector.tensor_copy(os_sb[:, :], pss[e][:, :])
                        nc.sync.dma_start(
                            out=out[s * MS : (s + 1) * MS, e * 512 : (e + 1) * 512],
                            in_=os_sb[:, :],
                        )

            emit_strip_eouter(2, conv_after_m=3)
            emit_strip_eouter(3)

    nc.compile()
    _BUILT["nc"] = nc
    return nc


def _shard_inputs(x, conv_w, conv_b, W_proj, b_proj):
    import ml_dtypes

    bf16 = ml_dtypes.bfloat16
    wt_np = np.ascontiguousarray(W_proj.T.astype(bf16))
    cw_np = np.ascontiguousarray(
        conv_w.reshape(KT, 128, KW).transpose(1, 0, 2).reshape(128, KT * KW),
        dtype=np.float32,
    )
    cb_np = np.ascontiguousarray(conv_b.reshape(KT, 128).T, dtype=np.float32)

    x16 = x.astype(bf16)
    per_batch = S // T
    in_maps = []
    for c in range(NCORES):
        b = c // per_batch
        s0 = (c % per_batch) * T
        xp = np.zeros((T + 3, D), dtype=bf16)
        xp[3:] = x16[b, s0 : s0 + T]
        if s0 > 0:
            xp[:3] = x16[b, s0 - 3 : s0]
        xTc = xp.T  # [D, T+3]
        # [NCS, D, CW+3] sliding strips -> [NCS, 16, 128, CW+3]
        strips = np.stack([xTc[:, i * CW : i * CW + CW + 3] for i in range(NCS)])
        strips = strips.reshape(NCS, KT, 128, CW + 3)
        # -> [NCS, 4 quarters, 128, 4*(CW+3)]
        quarters = np.ascontiguousarray(
            strips.reshape(NCS, KT // JQ, JQ, 128, CW + 3).transpose(0, 1, 3, 2, 4)
        ).reshape(NCS * (KT // JQ), 128, JQ * (CW + 3))
        xs0_np = np.ascontiguousarray(xTc[0:128, 0:260])
        xs0b_np = np.ascontiguousarray(xTc[0:128, 256:520])
        in_maps.append(
            {
                "xs_t": quarters,
                "xs0": xs0_np,
                "xs0b": xs0b_np,
                "wt": wt_np,
                "cw": cw_np,
                "cb": cb_np,
            }
        )
    return in_maps


def run_sharded(x, conv_w, conv_b, W_proj, b_proj, trace=False):
    """Run across the 8 cores; returns (full_out [B,S,D], BassKernelResults)."""
    from concourse.bass_utils import run_bass_kernel_spmd

    nc = _build_program()
    in_maps = _shard_inputs(x, conv_w, conv_b, W_proj, b_proj)
    try:
        res = run_bass_kernel_spmd(nc, in_maps, list(range(NCORES)), trace=trace)
    except Exception:
        # transient device wedges (NRT_EXEC_UNIT_UNRECOVERABLE) clear on retry
        res = run_bass_kernel_spmd(nc, in_maps, list(range(NCORES)), trace=trace)
    full = np.empty((B, S, D), dtype=np.float32)
    per_batch = S // T
    bp = b_proj.astype(np.float32)
    for c in range(NCORES):
        b = c // per_batch
        s0 = (c % per_batch) * T
        full[b, s0 : s0 + T] = res.results[c]["out"].astype(np.float32) + bp
    return full, res


def kernel(x, conv_w, conv_b, W_proj, b_proj):
    full, _ = run_sharded(x, conv_w, conv_b, W_proj, b_proj, trace=False)
    return full


# revision 50
# speedup vs baseline: 1.0082x; 1.0082x over previous
"""Trainium2 Bass kernel for nn_Expert_13082470383822.

y = silu(depthwise_causal_conv1d(x, conv_w, K=4) + conv_b);  out = y @ W_proj.T + b_proj
x [4, 4096, 2048] fp32. Data-parallel over the 16384 (batch*seq) tokens across
8 NeuronCores (2048 tokens/core + 3-token halo).

bf16 datapath. Per-core schedule:
- ~56 dummy N=64 matmuls at kernel start keep the PE busy through the HAM
  activity window so the real matmuls run at 2.4 GHz from the first issue.
- the projection runs j-outer phases of 8 PSUM banks covering ALL 4 m-tiles
  x 2 e-chunks of a 512-token strip; each W row-tile is split into column
  halves (wa = features 0-1023, wb = 1024-2047) so phase A only streams wa
  (half the W bandwidth demand of a full-tile phase) while wb arrives for
  phase B. Strips 0-2 are phase-based; strip 3 is e-outer per m-tile so the
  final copybacks overlap the remaining accumulation.
- each phase processes j14/j15 per bank-PAIR at staggered times so the 8
  PSUM copybacks (split 4 ACT / 4 DVE) pipeline with the next phase's
  restart instead of serializing against it.
- conv per j (full width): tap0 via DVE tensor_scalar, taps 2,3 via
  scalar_tensor_tensor, tap1 on ACT (Copy with per-partition scale), one
  DVE tensor_tensor combine, SiLU+conv_b on ACT. Conv for strip c+1 runs a
  full strip ahead of its phases.
- DMA: x0/x1 ride the ACT HWDGE queue (its triggers fire earliest), W
  halves + x3 + output stores on the sync queue in first-use order, consts
  + x2 on the SWDGE gpsimd queue. b_proj is added on the host.
"""

import sys

if "/opt/trn_rl_repo" not in sys.path:
    sys.path.insert(0, "/opt/trn_rl_repo")

import os

import numpy as np

if os.environ.get("BASS_LDW_OPT", "0") == "1":
    import concourse.bass_utils as _bu

    if not getattr(_bu, "_ldw_opt_patched", False):
        _orig_run_command = _bu.run_command

        def _run_command_ldw(cmd, *a, **kw):
            cmd = [
                "--enable-ldw-opt=true" if c == "--enable-ldw-opt=false" else c
                for c in cmd
            ]
            return _orig_run_command(cmd, *a, **kw)

        _bu.run_command = _run_command_ldw
        _bu._ldw_opt_patched = True

B, S, D, KW = 4, 4096, 2048, 4
NCORES = 8
T = (B * S) // NCORES  # tokens per core = 2048
KT = D // 128  # 16 channel tiles
ECH = D // 512  # 4 e-chunks of the output features
CW = 512  # conv strip width (tokens)
MS = 128  # matmul stationary strip width (tokens)
NCS = T // CW  # 4 conv strips
MPC = CW // MS  # 4 matmul strips per conv strip
JQ = 4  # j-tiles per x quarter-load
NDUM = 72  # warm-up matmuls

_BUILT = {}


def _build_program():
    if "nc" in _BUILT:
        return _BUILT["nc"]

    import concourse.tile as tile
    from concourse import bacc, mybir

    dt = mybir.dt
    AF = mybir.ActivationFunctionType
    ALU = mybir.AluOpType

    nc = bacc.Bacc("TRN2", target_bir_lowering=False, debug=False)
    # pre-tiled x: per (conv-strip, j-quarter): [128, 4*(CW+3)] bf16 contiguous
    xs_d = nc.declare_dram_parameter(
        "xs_t", [NCS * (KT // JQ), 128, JQ * (CW + 3)], dt.bfloat16, isOutput=False
    )
    # tiny j0-only head tiles of strip 0 (tokens -3..257 and 253..517) to
    # start conv early
    xs0_d = nc.declare_dram_parameter("xs0", [128, 260], dt.bfloat16, isOutput=False)
    xs0b_d = nc.declare_dram_parameter("xs0b", [128, 264], dt.bfloat16, isOutput=False)
    wt = nc.declare_dram_parameter("wt", [D, D], dt.bfloat16, isOutput=False)
    cw = nc.declare_dram_parameter("cw", [128, KT * KW], dt.float32, isOutput=False)
    cb = nc.declare_dram_parameter("cb", [128, KT], dt.float32, isOutput=False)
    out = nc.declare_dram_parameter("out", [T, D], dt.bfloat16, isOutput=True)

    with tile.TileContext(nc) as tc:
        with (
            tc.tile_pool(name="consts", bufs=1) as cpool,
            tc.tile_pool(name="wpool", bufs=1) as wpool,
            tc.tile_pool(name="xpool", bufs=12) as xpool,
            tc.tile_pool(name="ypool", bufs=3) as ypool,
            tc.tile_pool(name="apool", bufs=6) as apool,
            tc.tile_pool(name="opool", bufs=12) as opool,
            tc.tile_pool(name="pspool", bufs=8, space="PSUM") as pspool,
        ):
            xq = {}

            def load_xq(c, q, eng):
                xt = xpool.tile([128, JQ, CW + 3], dt.bfloat16, name="xs", tag="xs")
                eng.dma_start(
                    out=xt[:, :, :],
                    in_=xs_d[c * (KT // JQ) + q, :, :].rearrange(
                        "p (j t) -> p j t", j=JQ
                    ),
                )
                xq[(c, q)] = xt

            # ---- startup-critical x tiles ride the ACT HWDGE queue: its
            # triggers fire before the sync queue's preamble finishes; few
            # enough that the ring never blocks ACT compute ----
            xs0 = cpool.tile([128, 260], dt.bfloat16, name="xs0")
            nc.scalar.dma_start(out=xs0[:, :], in_=xs0_d[:, :])
            xs0b = cpool.tile([128, 264], dt.bfloat16, name="xs0b")
            nc.scalar.dma_start(out=xs0b[:, :], in_=xs0b_d[:, :])
            load_xq(0, 0, nc.scalar)
            load_xq(0, 1, nc.scalar)

            # ---- warm-up: ACT table + HAM via dummy matmuls ----
            dmw = cpool.tile([128, 64], dt.bfloat16, name="dmw")
            nc.vector.memset(dmw[:, :], 0.0)
            dum = cpool.tile([1, 1], dt.float32, name="dum")
            nc.vector.memset(dum[:, :], 0.0)
            nc.scalar.activation(dum[:, :], dum[:, :], AF.Silu, bias=0.0)
            dps = pspool.tile([128, 512], dt.float32, name="ps", tag="ps")
            for i in range(NDUM):
                nc.tensor.matmul(
                    dps[0:64, 0:64],
                    dmw[:, 0:64],
                    dmw[:, 0:64],
                    start=(i == 0),
                    stop=(i == NDUM - 1),
                )

            # ---- consts + the rest of x0, x1, x2 on the SWDGE gpsimd queue ----
            cw_sb = cpool.tile([128, KT * KW], dt.float32, name="cw_sb")
            nc.gpsimd.dma_start(out=cw_sb[:, :], in_=cw[:, :])
            cb_sb = cpool.tile([128, KT], dt.float32, name="cb_sb")
            nc.gpsimd.dma_start(out=cb_sb[:, :], in_=cb[:, :])
            load_xq(0, 2, nc.gpsimd)
            load_xq(0, 3, nc.gpsimd)
            for q in range(4):
                load_xq(1, q, nc.gpsimd)
            for q in range(4):
                load_xq(2, q, nc.gpsimd)

            # ---- W column-halves on sync in first-use order, then x3 ----
            wa = [None] * KT
            wb = [None] * KT
            for j in range(0, KT):
                wj = wpool.tile([128, 1024], dt.bfloat16, name=f"wa{j}")
                nc.sync.dma_start(
                    out=wj[:, :], in_=wt[j * 128 : (j + 1) * 128, 0:1024]
                )
                wa[j] = wj
            for j in range(KT):
                wj = wpool.tile([128, 1024], dt.bfloat16, name=f"wb{j}")
                nc.sync.dma_start(
                    out=wj[:, :], in_=wt[j * 128 : (j + 1) * 128, 1024:2048]
                )
                wb[j] = wj
            for q in range(4):
                load_xq(3, q, nc.sync)

            def wsel(j, e):
                w = wa[j] if e < 2 else wb[j]
                return w[:, (e % 2) * 512 : (e % 2) * 512 + 512]

            # ---- conv emission ----
            ys_strip = {}

            def conv_j0_half(h):
                """Strip-0 j0 in halves via pure-DVE chains (lowest latency:
                reads the tiny head tiles that land first)."""
                ys = ys_strip[0]
                tile0 = xs0 if h == 0 else xs0b
                t1 = apool.tile([128, 512], dt.bfloat16, name="t1", tag="t1")
                nc.vector.tensor_scalar(
                    t1[:, 0:256], tile0[:, 0:256], cw_sb[:, 0:1], None, ALU.mult
                )
                for k in range(1, KW):
                    nc.vector.scalar_tensor_tensor(
                        t1[:, 0:256], tile0[:, k : k + 256], cw_sb[:, k : k + 1],
                        t1[:, 0:256], ALU.mult, ALU.add,
                    )
                nc.scalar.activation(
                    ys[:, 0, h * 256 : h * 256 + 256], t1[:, 0:256], AF.Silu,
                    bias=cb_sb[:, 0:1],
                )

            def conv_j(c, j):
                xs = xq[(c, j // JQ)]
                jj = j % JQ
                ys = ys_strip[c]
                t2 = apool.tile([128, 512], dt.bfloat16, name="t2", tag="t2")
                nc.scalar.activation(
                    t2[:, :], xs[:, jj, 1 : 1 + CW], AF.Copy,
                    scale=cw_sb[:, j * KW + 1 : j * KW + 2],
                )
                t1 = apool.tile([128, 512], dt.bfloat16, name="t1", tag="t1")
                nc.vector.tensor_scalar(
                    t1[:, :], xs[:, jj, 0:CW], cw_sb[:, j * KW : j * KW + 1],
                    None, ALU.mult,
                )
                for k in (2, 3):
                    nc.vector.scalar_tensor_tensor(
                        t1[:, :], xs[:, jj, k : k + CW],
                        cw_sb[:, j * KW + k : j * KW + k + 1],
                        t1[:, :], ALU.mult, ALU.add,
                    )
                t3 = apool.tile([128, 512], dt.bfloat16, name="t3", tag="t3")
                nc.vector.tensor_tensor(t3[:, :], t1[:, :], t2[:, :], ALU.add)
                nc.scalar.activation(
                    ys[:, j, :], t3[:, :], AF.Silu, bias=cb_sb[:, j : j + 1]
                )

            # ---- matmul phases: all 4 m-tiles x one e-half, j-outer ----
            def mm_phase(c, eh):
                ys = ys_strip[c]
                pss = [
                    pspool.tile([128, 512], dt.float32, name="ps", tag="ps")
                    for _ in range(8)
                ]

                def mm(b, j):
                    m, el = b // 2, b % 2
                    nc.tensor.matmul(
                        pss[b][:, :],
                        ys[:, j, m * MS : (m + 1) * MS],
                        wsel(j, 2 * eh + el),
                        start=(j == 0),
                        stop=(j == KT - 1),
                    )

                # j0..13 round-robin; last two j per bank-PAIR staggered so
                # copybacks pipeline with the next phase's restarts
                for j in range(KT - 2):
                    for b in range(8):
                        mm(b, j)
                for bp in range(4):
                    for j in (KT - 2, KT - 1):
                        for b in (2 * bp, 2 * bp + 1):
                            mm(b, j)
                return pss

            def copyback(c, eh, pss):
                for b in range(8):
                    m, el = b // 2, b % 2
                    e = 2 * eh + el
                    s = c * MPC + m
                    os_sb = opool.tile([128, 512], dt.bfloat16, name="os", tag="os")
                    if b % 2 == 0:
                        nc.scalar.copy(os_sb[:, :], pss[b][:, :])
                    else:
                        nc.vector.tensor_copy(os_sb[:, :], pss[b][:, :])
                    nc.sync.dma_start(
                        out=out[s * MS : (s + 1) * MS, e * 512 : (e + 1) * 512],
                        in_=os_sb[:, :],
                    )

            # ================= schedule =================
            ys_strip[0] = ypool.tile([128, KT, CW], dt.bfloat16, name="ys", tag="ys")
            conv_j0_half(0)
            conv_j0_half(1)
            for j in range(1, KT):
                conv_j(0, j)

            pss_a = mm_phase(0, 0)

            ys_strip[1] = ypool.tile([128, KT, CW], dt.bfloat16, name="ys", tag="ys")
            conv_j(1, 0)
            copyback(0, 0, pss_a)

            pss_b = mm_phase(0, 1)

            for j in range(1, 12):
                conv_j(1, j)
            copyback(0, 1, pss_b)
            for j in range(12, KT):
                conv_j(1, j)

            pss_c = mm_phase(1, 0)

            ys_strip[2] = ypool.tile([128, KT, CW], dt.bfloat16, name="ys", tag="ys")
            for j in range(0, 8):
                conv_j(2, j)
            copyback(1, 0, pss_c)

            pss_d = mm_phase(1, 1)

            for j in range(8, KT):
                conv_j(2, j)
            ys_strip[3] = ypool.tile([128, KT, CW], dt.bfloat16, name="ys", tag="ys")
            for j in range(0, 5):
                conv_j(3, j)
            copyback(1, 1, pss_d)

            # strips 2+3: conv is a full strip ahead by now, so e-outer per m
            # (copybacks overlap the next accumulation, no phase boundaries);
            # strip-3 conv interleaves with strip-2's m-tiles
            s3_conv = iter(range(5, KT))

            def emit_strip_eouter(c, conv_after_m=0):
                ys = ys_strip[c]
                for m in range(MPC):
                    s = c * MPC + m
                    pss = []
                    for e in range(ECH):
                        ps = pspool.tile([128, 512], dt.float32, name="ps", tag="ps")
                        for j in range(KT):
                            nc.tensor.matmul(
                                ps[:, :],
                                ys[:, j, m * MS : (m + 1) * MS],
                                wsel(j, e),
                                start=(j == 0),
                                stop=(j == KT - 1),
                            )
                        pss.append(ps)
                    for _ in range(conv_after_m):
                        j3 = next(s3_conv, None)
                        if j3 is not None:
                            conv_j(3, j3)
                    last = c == NCS - 1 and m == MPC - 1
                    for e in range(ECH):
                        os_sb = opool.tile(
                            [128, 512], dt.bfloat16, name="os", tag="os"
                        )
                        if last and e >= ECH - 2:
                            # final tiles: split the copy across ACT+DVE to
                            # shave the drain tail
                            nc.scalar.copy(os_sb[:, 0:256], pss[e][:, 0:256])
                            nc.vector.tensor_copy(
                                os_sb[:, 256:512], pss[e][:, 256:512]
                            )
                        elif e % 2 == 0:
                            nc.scalar.copy(os_sb[:, :], pss[e][:, :])
                        else:
                            nc.vector.tensor_copy(os_sb[:, :], pss[e][:, :])
                        nc.sync.dma_start(
                            out=out[s * MS : (s + 1) * MS, e * 512 : (e + 1) * 512],
                            in_=os_sb[:, :],
                        )

            emit_strip_eouter(2, conv_after_m=3)
            emit_strip_eouter(3)

    nc.compile()
    _BUILT["nc"] = nc
    return nc


def _shard_inputs(x, conv_w, conv_b, W_proj, b_proj):
    import ml_dtypes

    bf16 = ml_dtypes.bfloat16
    wt_np = np.ascontiguousarray(W_proj.T.astype(bf16))
    cw_np = np.ascontiguousarray(
        conv_w.reshape(KT, 128, KW).transpose(1, 0, 2).reshape(128, KT * KW),
        dtype=np.float32,
    )
    cb_np = np.ascontiguousarray(conv_b.reshape(KT, 128).T, dtype=np.float32)

    x16 = x.astype(bf16)
    per_batch = S // T
    in_maps = []
    for c in range(NCORES):
        b = c // per_batch
        s0 = (c % per_batch) * T
        xp = np.zeros((T + 3, D), dtype=bf16)
        xp[3:] = x16[b, s0 : s0 + T]
        if s0 > 0:
            xp[:3] = x16[b, s0 - 3 : s0]
        xTc = xp.T  # [D, T+3]
        # [NCS, D, CW+3] sliding strips -> [NCS, 16, 128, CW+3]
        strips = np.stack([xTc[:, i * CW : i * CW + CW + 3] for i in range(NCS)])
        strips = strips.reshape(NCS, KT, 128, CW + 3)
        # -> [NCS, 4 quarters, 128, 4*(CW+3)]
        quarters = np.ascontiguousarray(
            strips.reshape(NCS, KT // JQ, JQ, 128, CW + 3).transpose(0, 1, 3, 2, 4)
        ).reshape(NCS * (KT // JQ), 128, JQ * (CW + 3))
        xs0_np = np.ascontiguousarray(xTc[0:128, 0:260])
        xs0b_np = np.ascontiguousarray(xTc[0:128, 256:520])
        in_maps.append(
            {
                "xs_t": quarters,
                "xs0": xs0_np,
                "xs0b": xs0b_np,
                "wt": wt_np,
                "cw": cw_np,
                "cb": cb_np,
            }
        )
    return in_maps


def run_sharded(x, conv_w, conv_b, W_proj, b_proj, trace=False):
    """Run across the 8 cores; returns (full_out [B,S,D], BassKernelResults)."""
    from concourse.bass_utils import run_bass_kernel_spmd

    nc = _build_program()
    in_maps = _shard_inputs(x, conv_w, conv_b, W_proj, b_proj)
    try:
        res = run_bass_kernel_spmd(nc, in_maps, list(range(NCORES)), trace=trace)
    except Exception:
        # transient device wedges (NRT_EXEC_UNIT_UNRECOVERABLE) clear on retry
        res = run_bass_kernel_spmd(nc, in_maps, list(range(NCORES)), trace=trace)
    full = np.empty((B, S, D), dtype=np.float32)
    per_batch = S // T
    bp = b_proj.astype(np.float32)
    for c in range(NCORES):
        b = c // per_batch
        s0 = (c % per_batch) * T
        full[b, s0 : s0 + T] = res.results[c]["out"].astype(np.float32) + bp
    return full, res


def kernel(x, conv_w, conv_b, W_proj, b_proj):
    full, _ = run_sharded(x, conv_w, conv_b, W_proj, b_proj, trace=False)
    return full


# revision 51
# speedup vs baseline: 1.0131x; 1.0049x over previous
"""Trainium2 Bass kernel for nn_Expert_13082470383822.

y = silu(depthwise_causal_conv1d(x, conv_w, K=4) + conv_b);  out = y @ W_proj.T + b_proj
x [4, 4096, 2048] fp32. Data-parallel over the 16384 (batch*seq) tokens across
8 NeuronCores (2048 tokens/core + 3-token halo).

bf16 datapath. Per-core schedule:
- 72 dummy N=64 matmuls at kernel start keep the PE busy through the HAM
  activity window so the real matmuls run at 2.4 GHz from the first issue.
- strips 0-1 run the projection as j-outer phases of 8 PSUM banks covering
  ALL 4 m-tiles x 2 e-chunks of a 512-token strip; each W row-tile is split
  into column halves (wa = features 0-1023, wb = 1024-2047) so phase A only
  streams wa (half the W bandwidth demand of a full-tile phase) while wb
  arrives for phase B. Strips 2-3 run e-outer per m-tile (their conv is a
  full strip ahead by then) so copybacks overlap the next accumulation.
- each phase processes j14/j15 per bank-PAIR at staggered times so the 8
  PSUM copybacks (split 4 ACT / 4 DVE) pipeline with the next phase's
  restart instead of serializing against it.
- conv per j (full width): tap0 via DVE tensor_scalar, taps 2,3 via
  scalar_tensor_tensor, tap1 on ACT (Copy with per-partition scale), one
  DVE tensor_tensor combine, SiLU+conv_b on ACT. Strip-0 j0 is computed in
  halves from two tiny head tiles so the first matmul issues early.
- DMA: the startup-critical tiles (j0 head tiles + first x0 quarters) ride
  the ACT HWDGE queue, whose triggers fire earliest (only 4 — more blocks
  ACT compute on the ring); W halves + x3 + output stores on the sync HWDGE
  queue in first-use order; consts + the rest of x0, x1, x2 on the SWDGE
  gpsimd queue. b_proj is added on the host.
"""

import sys

if "/opt/trn_rl_repo" not in sys.path:
    sys.path.insert(0, "/opt/trn_rl_repo")

import os

import numpy as np

if os.environ.get("BASS_LDW_OPT", "0") == "1":
    import concourse.bass_utils as _bu

    if not getattr(_bu, "_ldw_opt_patched", False):
        _orig_run_command = _bu.run_command

        def _run_command_ldw(cmd, *a, **kw):
            cmd = [
                "--enable-ldw-opt=true" if c == "--enable-ldw-opt=false" else c
                for c in cmd
            ]
            return _orig_run_command(cmd, *a, **kw)

        _bu.run_command = _run_command_ldw
        _bu._ldw_opt_patched = True

B, S, D, KW = 4, 4096, 2048, 4
NCORES = 8
T = (B * S) // NCORES  # tokens per core = 2048
KT = D // 128  # 16 channel tiles
ECH = D // 512  # 4 e-chunks of the output features
CW = 512  # conv strip width (tokens)
MS = 128  # matmul stationary strip width (tokens)
NCS = T // CW  # 4 conv strips
MPC = CW // MS  # 4 matmul strips per conv strip
JQ = 4  # j-tiles per x quarter-load
NDUM = 72  # warm-up matmuls

_BUILT = {}


def _build_program():
    if "nc" in _BUILT:
        return _BUILT["nc"]

    import concourse.tile as tile
    from concourse import bacc, mybir

    dt = mybir.dt
    AF = mybir.ActivationFunctionType
    ALU = mybir.AluOpType

    nc = bacc.Bacc("TRN2", target_bir_lowering=False, debug=False)
    # pre-tiled x: per (conv-strip, j-quarter): [128, 4*(CW+3)] bf16 contiguous
    xs_d = nc.declare_dram_parameter(
        "xs_t", [NCS * (KT // JQ), 128, JQ * (CW + 3)], dt.bfloat16, isOutput=False
    )
    # tiny j0-only head tiles of strip 0 (tokens -3..257 and 253..517) to
    # start conv early
    xs0_d = nc.declare_dram_parameter("xs0", [128, 260], dt.bfloat16, isOutput=False)
    xs0b_d = nc.declare_dram_parameter("xs0b", [128, 264], dt.bfloat16, isOutput=False)
    wt = nc.declare_dram_parameter("wt", [D, D], dt.bfloat16, isOutput=False)
    cw = nc.declare_dram_parameter("cw", [128, KT * KW], dt.float32, isOutput=False)
    cb = nc.declare_dram_parameter("cb", [128, KT], dt.float32, isOutput=False)
    out = nc.declare_dram_parameter("out", [T, D], dt.bfloat16, isOutput=True)

    with tile.TileContext(nc) as tc:
        with (
            tc.tile_pool(name="consts", bufs=1) as cpool,
            tc.tile_pool(name="wpool", bufs=1) as wpool,
            tc.tile_pool(name="xpool", bufs=12) as xpool,
            tc.tile_pool(name="ypool", bufs=3) as ypool,
            tc.tile_pool(name="apool", bufs=6) as apool,
            tc.tile_pool(name="opool", bufs=12) as opool,
            tc.tile_pool(name="pspool", bufs=8, space="PSUM") as pspool,
        ):
            xq = {}

            def load_xq(c, q, eng):
                xt = xpool.tile([128, JQ, CW + 3], dt.bfloat16, name="xs", tag="xs")
                eng.dma_start(
                    out=xt[:, :, :],
                    in_=xs_d[c * (KT // JQ) + q, :, :].rearrange(
                        "p (j t) -> p j t", j=JQ
                    ),
                )
                xq[(c, q)] = xt

            # ---- startup-critical x tiles ride the ACT HWDGE queue: its
            # triggers fire before the sync queue's preamble finishes; few
            # enough that the ring never blocks ACT compute ----
            xs0 = cpool.tile([128, 260], dt.bfloat16, name="xs0")
            nc.scalar.dma_start(out=xs0[:, :], in_=xs0_d[:, :])
            xs0b = cpool.tile([128, 264], dt.bfloat16, name="xs0b")
            nc.scalar.dma_start(out=xs0b[:, :], in_=xs0b_d[:, :])
            load_xq(0, 0, nc.scalar)
            load_xq(0, 1, nc.scalar)

            # ---- warm-up: ACT table + HAM via dummy matmuls ----
            dmw = cpool.tile([128, 64], dt.bfloat16, name="dmw")
            nc.vector.memset(dmw[:, :], 0.0)
            dum = cpool.tile([1, 1], dt.float32, name="dum")
            nc.vector.memset(dum[:, :], 0.0)
            nc.scalar.activation(dum[:, :], dum[:, :], AF.Silu, bias=0.0)
            dps = pspool.tile([128, 512], dt.float32, name="ps", tag="ps")
            for i in range(NDUM):
                nc.tensor.matmul(
                    dps[0:64, 0:64],
                    dmw[:, 0:64],
                    dmw[:, 0:64],
                    start=(i == 0),
                    stop=(i == NDUM - 1),
                )

            # ---- consts + the rest of x0, x1, x2 on the SWDGE gpsimd queue ----
            cw_sb = cpool.tile([128, KT * KW], dt.float32, name="cw_sb")
            nc.gpsimd.dma_start(out=cw_sb[:, :], in_=cw[:, :])
            cb_sb = cpool.tile([128, KT], dt.float32, name="cb_sb")
            nc.gpsimd.dma_start(out=cb_sb[:, :], in_=cb[:, :])
            load_xq(0, 2, nc.gpsimd)
            load_xq(0, 3, nc.gpsimd)
            for q in range(4):
                load_xq(1, q, nc.gpsimd)
            for q in range(4):
                load_xq(2, q, nc.gpsimd)

            # ---- W column-halves on sync in first-use order, then x3 ----
            wa = [None] * KT
            wb = [None] * KT
            for j in range(0, KT):
                wj = wpool.tile([128, 1024], dt.bfloat16, name=f"wa{j}")
                nc.sync.dma_start(
                    out=wj[:, :], in_=wt[j * 128 : (j + 1) * 128, 0:1024]
                )
                wa[j] = wj
            for j in range(KT):
                wj = wpool.tile([128, 1024], dt.bfloat16, name=f"wb{j}")
                nc.sync.dma_start(
                    out=wj[:, :], in_=wt[j * 128 : (j + 1) * 128, 1024:2048]
                )
                wb[j] = wj
            for q in range(4):
                load_xq(3, q, nc.sync)

            def wsel(j, e):
                w = wa[j] if e < 2 else wb[j]
                return w[:, (e % 2) * 512 : (e % 2) * 512 + 512]

            # ---- conv emission ----
            ys_strip = {}

            def conv_j0_half(h):
                """Strip-0 j0 in halves via pure-DVE chains (lowest latency:
                reads the tiny head tiles that land first)."""
                ys = ys_strip[0]
                tile0 = xs0 if h == 0 else xs0b
                t1 = apool.tile([128, 512], dt.bfloat16, name="t1", tag="t1")
                nc.vector.tensor_scalar(
                    t1[:, 0:256], tile0[:, 0:256], cw_sb[:, 0:1], None, ALU.mult
                )
                for k in range(1, KW):
                    nc.vector.scalar_tensor_tensor(
                        t1[:, 0:256], tile0[:, k : k + 256], cw_sb[:, k : k + 1],
                        t1[:, 0:256], ALU.mult, ALU.add,
                    )
                nc.scalar.activation(
                    ys[:, 0, h * 256 : h * 256 + 256], t1[:, 0:256], AF.Silu,
                    bias=cb_sb[:, 0:1],
                )

            def conv_j(c, j):
                xs = xq[(c, j // JQ)]
                jj = j % JQ
                ys = ys_strip[c]
                t2 = apool.tile([128, 512], dt.bfloat16, name="t2", tag="t2")
                nc.scalar.activation(
                    t2[:, :], xs[:, jj, 1 : 1 + CW], AF.Copy,
                    scale=cw_sb[:, j * KW + 1 : j * KW + 2],
                )
                t1 = apool.tile([128, 512], dt.bfloat16, name="t1", tag="t1")
                nc.vector.tensor_scalar(
                    t1[:, :], xs[:, jj, 0:CW], cw_sb[:, j * KW : j * KW + 1],
                    None, ALU.mult,
                )
                for k in (2, 3):
                    nc.vector.scalar_tensor_tensor(
                        t1[:, :], xs[:, jj, k : k + CW],
                        cw_sb[:, j * KW + k : j * KW + k + 1],
                        t1[:, :], ALU.mult, ALU.add,
                    )
                t3 = apool.tile([128, 512], dt.bfloat16, name="t3", tag="t3")
                nc.vector.tensor_tensor(t3[:, :], t1[:, :], t2[:, :], ALU.add)
                nc.scalar.activation(
                    ys[:, j, :], t3[:, :], AF.Silu, bias=cb_sb[:, j : j + 1]
                )

            # ---- matmul phases: all 4 m-tiles x one e-half, j-outer ----
            def mm_phase(c, eh):
                ys = ys_strip[c]
                pss = [
                    pspool.tile([128, 512], dt.float32, name="ps", tag="ps")
                    for _ in range(8)
                ]

                def mm(b, j):
                    m, el = b // 2, b % 2
                    nc.tensor.matmul(
                        pss[b][:, :],
                        ys[:, j, m * MS : (m + 1) * MS],
                        wsel(j, 2 * eh + el),
                        start=(j == 0),
                        stop=(j == KT - 1),
                    )

                # j0..13 round-robin; last two j per bank-PAIR staggered so
                # copybacks pipeline with the next phase's restarts
                for j in range(KT - 2):
                    for b in range(8):
                        mm(b, j)
                for bp in range(4):
                    for j in (KT - 2, KT - 1):
                        for b in (2 * bp, 2 * bp + 1):
                            mm(b, j)
                return pss

            def copyback(c, eh, pss):
                for b in range(8):
                    m, el = b // 2, b % 2
                    e = 2 * eh + el
                    s = c * MPC + m
                    os_sb = opool.tile([128, 512], dt.bfloat16, name="os", tag="os")
                    if b % 2 == 0:
                        nc.scalar.copy(os_sb[:, :], pss[b][:, :])
                    else:
                        nc.vector.tensor_copy(os_sb[:, :], pss[b][:, :])
                    nc.sync.dma_start(
                        out=out[s * MS : (s + 1) * MS, e * 512 : (e + 1) * 512],
                        in_=os_sb[:, :],
                    )

            # ================= schedule =================
            ys_strip[0] = ypool.tile([128, KT, CW], dt.bfloat16, name="ys", tag="ys")
            conv_j0_half(0)
            conv_j0_half(1)
            for j in range(1, KT):
                conv_j(0, j)

            pss_a = mm_phase(0, 0)

            ys_strip[1] = ypool.tile([128, KT, CW], dt.bfloat16, name="ys", tag="ys")
            conv_j(1, 0)
            copyback(0, 0, pss_a)

            pss_b = mm_phase(0, 1)

            for j in range(1, 12):
                conv_j(1, j)
            copyback(0, 1, pss_b)
            for j in range(12, KT):
                conv_j(1, j)

            pss_c = mm_phase(1, 0)

            ys_strip[2] = ypool.tile([128, KT, CW], dt.bfloat16, name="ys", tag="ys")
            for j in range(0, 8):
                conv_j(2, j)
            copyback(1, 0, pss_c)

            pss_d = mm_phase(1, 1)

            for j in range(8, KT):
                conv_j(2, j)
            ys_strip[3] = ypool.tile([128, KT, CW], dt.bfloat16, name="ys", tag="ys")
            for j in range(0, 5):
                conv_j(3, j)
            copyback(1, 1, pss_d)

            # strips 2+3: conv is a full strip ahead by now, so e-outer per m
            # (copybacks overlap the next accumulation, no phase boundaries);
            # strip-3 conv interleaves with strip-2's m-tiles
            s3_conv = iter(range(5, KT))

            def emit_strip_eouter(c, conv_after_m=0):
                ys = ys_strip[c]
                for m in range(MPC):
                    s = c * MPC + m
                    pss = []
                    for e in range(ECH):
                        ps = pspool.tile([128, 512], dt.float32, name="ps", tag="ps")
                        for j in range(KT):
                            nc.tensor.matmul(
                                ps[:, :],
                                ys[:, j, m * MS : (m + 1) * MS],
                                wsel(j, e),
                                start=(j == 0),
                                stop=(j == KT - 1),
                            )
                        pss.append(ps)
                    for _ in range(conv_after_m):
                        j3 = next(s3_conv, None)
                        if j3 is not None:
                            conv_j(3, j3)
                    last = c == NCS - 1 and m == MPC - 1
                    for e in range(ECH):
                        os_sb = opool.tile(
                            [128, 512], dt.bfloat16, name="os", tag="os"
                        )
                        if last and e >= ECH - 2:
                            # final tiles: split the copy across ACT+DVE to
                            # shave the drain tail
                            nc.scalar.copy(os_sb[:, 0:256], pss[e][:, 0:256])
                            nc.vector.tensor_copy(
                                os_sb[:, 256:512], pss[e][:, 256:512]
                            )
                        elif e % 2 == 0:
                            nc.scalar.copy(os_sb[:, :], pss[e][:, :])
                        else:
                            nc.vector.tensor_copy(os_sb[:, :], pss[e][:, :])
                        nc.sync.dma_start(
                            out=out[s * MS : (s + 1) * MS, e * 512 : (e + 1) * 512],
                            in_=os_sb[:, :],
                        )

            emit_strip_eouter(2, conv_after_m=3)
            emit_strip_eouter(3)

    nc.compile()
    _BUILT["nc"] = nc
    return nc


def _shard_inputs(x, conv_w, conv_b, W_proj, b_proj):
    import ml_dtypes

    bf16 = ml_dtypes.bfloat16
    wt_np = np.ascontiguousarray(W_proj.T.astype(bf16))
    cw_np = np.ascontiguousarray(
        conv_w.reshape(KT, 128, KW).transpose(1, 0, 2).reshape(128, KT * KW),
        dtype=np.float32,
    )
    cb_np = np.ascontiguousarray(conv_b.reshape(KT, 128).T, dtype=np.float32)

    x16 = x.astype(bf16)
    per_batch = S // T
    in_maps = []
    for c in range(NCORES):
        b = c // per_batch
        s0 = (c % per_batch) * T
        xp = np.zeros((T + 3, D), dtype=bf16)
        xp[3:] = x16[b, s0 : s0 + T]
        if s0 > 0:
            xp[:3] = x16[b, s0 - 3 : s0]
        xTc = xp.T  # [D, T+3]
        # [NCS, D, CW+3] sliding strips -> [NCS, 16, 128, CW+3]
        strips = np.stack([xTc[:, i * CW : i * CW + CW + 3] for i in range(NCS)])
        strips = strips.reshape(NCS, KT, 128, CW + 3)
        # -> [NCS, 4 quarters, 128, 4*(CW+3)]
        quarters = np.ascontiguousarray(
            strips.reshape(NCS, KT // JQ, JQ, 128, CW + 3).transpose(0, 1, 3, 2, 4)
        ).reshape(NCS * (KT // JQ), 128, JQ * (CW + 3))
        xs0_np = np.ascontiguousarray(xTc[0:128, 0:260])
        xs0b_np = np.ascontiguousarray(xTc[0:128, 256:520])
        in_maps.append(
            {
                "xs_t": quarters,
                "xs0": xs0_np,
                "xs0b": xs0b_np,
                "wt": wt_np,
                "cw": cw_np,
                "cb": cb_np,
            }
        )
    return in_maps


def run_sharded(x, conv_w, conv_b, W_proj, b_proj, trace=False):
    """Run across the 8 cores; returns (full_out [B,S,D], BassKernelResults)."""
    from concourse.bass_utils import run_bass_kernel_spmd

    nc = _build_program()
    in_maps = _shard_inputs(x, conv_w, conv_b, W_proj, b_proj)
    try:
        res = run_bass_kernel_spmd(nc, in_maps, list(range(NCORES)), trace=trace)
    except Exception:
        # transient device wedges (NRT_EXEC_UNIT_UNRECOVERABLE) clear on retry
        res = run_bass_kernel_spmd(nc, in_maps, list(range(NCORES)), trace=trace)
    full = np.empty((B, S, D), dtype=np.float32)
    per_batch = S // T
    bp = b_proj.astype(np.float32)
    for c in range(NCORES):
        b = c // per_batch
        s0 = (c % per_batch) * T
        full[b, s0 : s0 + T] = res.results[c]["out"].astype(np.float32) + bp
    return full, res


def kernel(x, conv_w, conv_b, W_proj, b_proj):
    full, _ = run_sharded(x, conv_w, conv_b, W_proj, b_proj, trace=False)
    return full


# revision 52
# speedup vs baseline: 1.0158x; 1.0026x over previous
"""Trainium2 Bass kernel for nn_Expert_13082470383822.

y = silu(depthwise_causal_conv1d(x, conv_w, K=4) + conv_b);  out = y @ W_proj.T + b_proj
x [4, 4096, 2048] fp32. Data-parallel over the 16384 (batch*seq) tokens across
8 NeuronCores (2048 tokens/core + 3-token halo).

bf16 datapath. Per-core schedule:
- 72 dummy N=64 matmuls at kernel start keep the PE busy through the HAM
  activity window so the real matmuls run at 2.4 GHz from the first issue.
- strips 0-1 run the projection as j-outer phases of 8 PSUM banks covering
  ALL 4 m-tiles x 2 e-chunks of a 512-token strip; each W row-tile is split
  into column halves (wa = features 0-1023, wb = 1024-2047) so phase A only
  streams wa (half the W bandwidth demand of a full-tile phase) while wb
  arrives for phase B. Strips 2-3 run e-outer per m-tile (their conv is a
  full strip ahead by then) so copybacks overlap the next accumulation.
- each phase processes j14/j15 per bank-PAIR at staggered times so the 8
  PSUM copybacks (split 4 ACT / 4 DVE) pipeline with the next phase's
  restart instead of serializing against it.
- conv per j (full width): tap0 via DVE tensor_scalar, taps 2,3 via
  scalar_tensor_tensor, tap1 on ACT (Copy with per-partition scale), one
  DVE tensor_tensor combine, SiLU+conv_b on ACT. Strip-0 j0 is computed in
  halves from two tiny head tiles so the first matmul issues early.
- DMA: the startup-critical tiles (j0 head tiles + first x0 quarters) ride
  the ACT HWDGE queue, whose triggers fire earliest (only 4 — more blocks
  ACT compute on the ring); W halves + x3 + output stores on the sync HWDGE
  queue in first-use order; consts + the rest of x0, x1, x2 on the SWDGE
  gpsimd queue. b_proj is added on the host.
"""

import sys

if "/opt/trn_rl_repo" not in sys.path:
    sys.path.insert(0, "/opt/trn_rl_repo")

import os

import numpy as np

if os.environ.get("BASS_LDW_OPT", "0") == "1":
    import concourse.bass_utils as _bu

    if not getattr(_bu, "_ldw_opt_patched", False):
        _orig_run_command = _bu.run_command

        def _run_command_ldw(cmd, *a, **kw):
            cmd = [
                "--enable-ldw-opt=true" if c == "--enable-ldw-opt=false" else c
                for c in cmd
            ]
            return _orig_run_command(cmd, *a, **kw)

        _bu.run_command = _run_command_ldw
        _bu._ldw_opt_patched = True

B, S, D, KW = 4, 4096, 2048, 4
NCORES = 8
T = (B * S) // NCORES  # tokens per core = 2048
KT = D // 128  # 16 channel tiles
ECH = D // 512  # 4 e-chunks of the output features
CW = 512  # conv strip width (tokens)
MS = 128  # matmul stationary strip width (tokens)
NCS = T // CW  # 4 conv strips
MPC = CW // MS  # 4 matmul strips per conv strip
JQ = 4  # j-tiles per x quarter-load
NDUM = 72  # warm-up matmuls

_BUILT = {}


def _build_program():
    if "nc" in _BUILT:
        return _BUILT["nc"]

    import concourse.tile as tile
    from concourse import bacc, mybir

    dt = mybir.dt
    AF = mybir.ActivationFunctionType
    ALU = mybir.AluOpType

    nc = bacc.Bacc("TRN2", target_bir_lowering=False, debug=False)
    # pre-tiled x: per (conv-strip, j-quarter): [128, 4*(CW+3)] bf16 contiguous
    xs_d = nc.declare_dram_parameter(
        "xs_t", [NCS * (KT // JQ), 128, JQ * (CW + 3)], dt.bfloat16, isOutput=False
    )
    # tiny j0-only head tiles of strip 0 (tokens -3..257 and 253..517) to
    # start conv early
    xs0_d = nc.declare_dram_parameter("xs0", [128, 260], dt.bfloat16, isOutput=False)
    xs0b_d = nc.declare_dram_parameter("xs0b", [128, 264], dt.bfloat16, isOutput=False)
    wt = nc.declare_dram_parameter("wt", [D, D], dt.bfloat16, isOutput=False)
    cw = nc.declare_dram_parameter("cw", [128, KT * KW], dt.float32, isOutput=False)
    cb = nc.declare_dram_parameter("cb", [128, KT], dt.float32, isOutput=False)
    out = nc.declare_dram_parameter("out", [T, D], dt.bfloat16, isOutput=True)

    with tile.TileContext(nc) as tc:
        with (
            tc.tile_pool(name="consts", bufs=1) as cpool,
            tc.tile_pool(name="wpool", bufs=1) as wpool,
            tc.tile_pool(name="xpool", bufs=12) as xpool,
            tc.tile_pool(name="ypool", bufs=3) as ypool,
            tc.tile_pool(name="apool", bufs=6) as apool,
            tc.tile_pool(name="opool", bufs=12) as opool,
            tc.tile_pool(name="pspool", bufs=8, space="PSUM") as pspool,
        ):
            xq = {}

            def load_xq(c, q, eng):
                xt = xpool.tile([128, JQ, CW + 3], dt.bfloat16, name="xs", tag="xs")
                eng.dma_start(
                    out=xt[:, :, :],
                    in_=xs_d[c * (KT // JQ) + q, :, :].rearrange(
                        "p (j t) -> p j t", j=JQ
                    ),
                )
                xq[(c, q)] = xt

            # ---- startup-critical x tiles ride the ACT HWDGE queue: its
            # triggers fire before the sync queue's preamble finishes; few
            # enough that the ring never blocks ACT compute. x0q0 first: it
            # paces strip-0's conv (and hence all of phase A) ----
            load_xq(0, 0, nc.scalar)
            xs0 = cpool.tile([128, 260], dt.bfloat16, name="xs0")
            nc.scalar.dma_start(out=xs0[:, :], in_=xs0_d[:, :])
            xs0b = cpool.tile([128, 264], dt.bfloat16, name="xs0b")
            nc.scalar.dma_start(out=xs0b[:, :], in_=xs0b_d[:, :])
            load_xq(0, 1, nc.scalar)

            # ---- warm-up: ACT table + HAM via dummy matmuls ----
            dmw = cpool.tile([128, 64], dt.bfloat16, name="dmw")
            nc.vector.memset(dmw[:, :], 0.0)
            dum = cpool.tile([1, 1], dt.float32, name="dum")
            nc.vector.memset(dum[:, :], 0.0)
            nc.scalar.activation(dum[:, :], dum[:, :], AF.Silu, bias=0.0)
            dps = pspool.tile([128, 512], dt.float32, name="ps", tag="ps")
            for i in range(NDUM):
                nc.tensor.matmul(
                    dps[0:64, 0:64],
                    dmw[:, 0:64],
                    dmw[:, 0:64],
                    start=(i == 0),
                    stop=(i == NDUM - 1),
                )

            # ---- consts + the rest of x0, x1, x2 on the SWDGE gpsimd queue ----
            cw_sb = cpool.tile([128, KT * KW], dt.float32, name="cw_sb")
            nc.gpsimd.dma_start(out=cw_sb[:, :], in_=cw[:, :])
            cb_sb = cpool.tile([128, KT], dt.float32, name="cb_sb")
            nc.gpsimd.dma_start(out=cb_sb[:, :], in_=cb[:, :])
            load_xq(0, 2, nc.gpsimd)
            load_xq(0, 3, nc.gpsimd)
            for q in range(4):
                load_xq(1, q, nc.gpsimd)
            for q in range(4):
                load_xq(2, q, nc.gpsimd)

            # ---- W column-halves on sync in first-use order, then x3 ----
            wa = [None] * KT
            wb = [None] * KT
            for j in range(0, KT):
                wj = wpool.tile([128, 1024], dt.bfloat16, name=f"wa{j}")
                nc.sync.dma_start(
                    out=wj[:, :], in_=wt[j * 128 : (j + 1) * 128, 0:1024]
                )
                wa[j] = wj
            for j in range(KT):
                wj = wpool.tile([128, 1024], dt.bfloat16, name=f"wb{j}")
                nc.sync.dma_start(
                    out=wj[:, :], in_=wt[j * 128 : (j + 1) * 128, 1024:2048]
                )
                wb[j] = wj
            for q in range(4):
                load_xq(3, q, nc.sync)

            def wsel(j, e):
                w = wa[j] if e < 2 else wb[j]
                return w[:, (e % 2) * 512 : (e % 2) * 512 + 512]

            # ---- conv emission ----
            ys_strip = {}

            def conv_j0_half(h):
                """Strip-0 j0 in halves via pure-DVE chains (lowest latency:
                reads the tiny head tiles that land first)."""
                ys = ys_strip[0]
                tile0 = xs0 if h == 0 else xs0b
                t1 = apool.tile([128, 512], dt.bfloat16, name="t1", tag="t1")
                nc.vector.tensor_scalar(
                    t1[:, 0:256], tile0[:, 0:256], cw_sb[:, 0:1], None, ALU.mult
                )
                for k in range(1, KW):
                    nc.vector.scalar_tensor_tensor(
                        t1[:, 0:256], tile0[:, k : k + 256], cw_sb[:, k : k + 1],
                        t1[:, 0:256], ALU.mult, ALU.add,
                    )
                nc.scalar.activation(
                    ys[:, 0, h * 256 : h * 256 + 256], t1[:, 0:256], AF.Silu,
                    bias=cb_sb[:, 0:1],
                )

            def conv_j(c, j):
                xs = xq[(c, j // JQ)]
                jj = j % JQ
                ys = ys_strip[c]
                t2 = apool.tile([128, 512], dt.bfloat16, name="t2", tag="t2")
                nc.scalar.activation(
                    t2[:, :], xs[:, jj, 1 : 1 + CW], AF.Copy,
                    scale=cw_sb[:, j * KW + 1 : j * KW + 2],
                )
                t1 = apool.tile([128, 512], dt.bfloat16, name="t1", tag="t1")
                nc.vector.tensor_scalar(
                    t1[:, :], xs[:, jj, 0:CW], cw_sb[:, j * KW : j * KW + 1],
                    None, ALU.mult,
                )
                for k in (2, 3):
                    nc.vector.scalar_tensor_tensor(
                        t1[:, :], xs[:, jj, k : k + CW],
                        cw_sb[:, j * KW + k : j * KW + k + 1],
                        t1[:, :], ALU.mult, ALU.add,
                    )
                t3 = apool.tile([128, 512], dt.bfloat16, name="t3", tag="t3")
                nc.vector.tensor_tensor(t3[:, :], t1[:, :], t2[:, :], ALU.add)
                nc.scalar.activation(
                    ys[:, j, :], t3[:, :], AF.Silu, bias=cb_sb[:, j : j + 1]
                )

            # ---- matmul phases: all 4 m-tiles x one e-half, j-outer ----
            def mm_phase(c, eh):
                ys = ys_strip[c]
                pss = [
                    pspool.tile([128, 512], dt.float32, name="ps", tag="ps")
                    for _ in range(8)
                ]

                def mm(b, j):
                    m, el = b // 2, b % 2
                    nc.tensor.matmul(
                        pss[b][:, :],
                        ys[:, j, m * MS : (m + 1) * MS],
                        wsel(j, 2 * eh + el),
                        start=(j == 0),
                        stop=(j == KT - 1),
                    )

                # j0..13 round-robin; last two j per bank-PAIR staggered so
                # copybacks pipeline with the next phase's restarts
                for j in range(KT - 2):
                    for b in range(8):
                        mm(b, j)
                for bp in range(4):
                    for j in (KT - 2, KT - 1):
                        for b in (2 * bp, 2 * bp + 1):
                            mm(b, j)
                return pss

            def copyback(c, eh, pss):
                for b in range(8):
                    m, el = b // 2, b % 2
                    e = 2 * eh + el
                    s = c * MPC + m
                    os_sb = opool.tile([128, 512], dt.bfloat16, name="os", tag="os")
                    if b % 2 == 0:
                        nc.scalar.copy(os_sb[:, :], pss[b][:, :])
                    else:
                        nc.vector.tensor_copy(os_sb[:, :], pss[b][:, :])
                    nc.sync.dma_start(
                        out=out[s * MS : (s + 1) * MS, e * 512 : (e + 1) * 512],
                        in_=os_sb[:, :],
                    )

            # ================= schedule =================
            ys_strip[0] = ypool.tile([128, KT, CW], dt.bfloat16, name="ys", tag="ys")
            conv_j0_half(0)
            conv_j0_half(1)
            for j in range(1, KT):
                conv_j(0, j)

            pss_a = mm_phase(0, 0)

            ys_strip[1] = ypool.tile([128, KT, CW], dt.bfloat16, name="ys", tag="ys")
            conv_j(1, 0)
            copyback(0, 0, pss_a)

            pss_b = mm_phase(0, 1)

            for j in range(1, 12):
                conv_j(1, j)
            copyback(0, 1, pss_b)
            for j in range(12, KT):
                conv_j(1, j)

            pss_c = mm_phase(1, 0)

            ys_strip[2] = ypool.tile([128, KT, CW], dt.bfloat16, name="ys", tag="ys")
            for j in range(0, 8):
                conv_j(2, j)
            copyback(1, 0, pss_c)

            pss_d = mm_phase(1, 1)

            for j in range(8, KT):
                conv_j(2, j)
            ys_strip[3] = ypool.tile([128, KT, CW], dt.bfloat16, name="ys", tag="ys")
            for j in range(0, 5):
                conv_j(3, j)
            copyback(1, 1, pss_d)

            # strips 2+3: conv is a full strip ahead by now, so e-outer per m
            # (copybacks overlap the next accumulation, no phase boundaries);
            # strip-3 conv interleaves with strip-2's m-tiles
            s3_conv = iter(range(5, KT))

            def emit_strip_eouter(c, conv_after_m=0):
                ys = ys_strip[c]
                for m in range(MPC):
                    s = c * MPC + m
                    pss = []
                    for e in range(ECH):
                        ps = pspool.tile([128, 512], dt.float32, name="ps", tag="ps")
                        for j in range(KT):
                            nc.tensor.matmul(
                                ps[:, :],
                                ys[:, j, m * MS : (m + 1) * MS],
                                wsel(j, e),
                                start=(j == 0),
                                stop=(j == KT - 1),
                            )
                        pss.append(ps)
                    for _ in range(conv_after_m):
                        j3 = next(s3_conv, None)
                        if j3 is not None:
                            conv_j(3, j3)
                    last = c == NCS - 1 and m == MPC - 1
                    for e in range(ECH):
                        os_sb = opool.tile(
                            [128, 512], dt.bfloat16, name="os", tag="os"
                        )
                        if last and e >= ECH - 2:
                            # final tiles: split the copy across ACT+DVE to
                            # shave the drain tail
                            nc.scalar.copy(os_sb[:, 0:256], pss[e][:, 0:256])
                            nc.vector.tensor_copy(
                                os_sb[:, 256:512], pss[e][:, 256:512]
                            )
                        elif e % 2 == 0:
                            nc.scalar.copy(os_sb[:, :], pss[e][:, :])
                        else:
                            nc.vector.tensor_copy(os_sb[:, :], pss[e][:, :])
                        nc.sync.dma_start(
                            out=out[s * MS : (s + 1) * MS, e * 512 : (e + 1) * 512],
                            in_=os_sb[:, :],
                        )

            emit_strip_eouter(2, conv_after_m=3)
            emit_strip_eouter(3)

    nc.compile()
    _BUILT["nc"] = nc
    return nc


def _shard_inputs(x, conv_w, conv_b, W_proj, b_proj):
    import ml_dtypes

    bf16 = ml_dtypes.bfloat16
    wt_np = np.ascontiguousarray(W_proj.T.astype(bf16))
    cw_np = np.ascontiguousarray(
        conv_w.reshape(KT, 128, KW).transpose(1, 0, 2).reshape(128, KT * KW),
        dtype=np.float32,
    )
    cb_np = np.ascontiguousarray(conv_b.reshape(KT, 128).T, dtype=np.float32)

    x16 = x.astype(bf16)
    per_batch = S // T
    in_maps = []
    for c in range(NCORES):
        b = c // per_batch
        s0 = (c % per_batch) * T
        xp = np.zeros((T + 3, D), dtype=bf16)
        xp[3:] = x16[b, s0 : s0 + T]
        if s0 > 0:
            xp[:3] = x16[b, s0 - 3 : s0]
        xTc = xp.T  # [D, T+3]
        # [NCS, D, CW+3] sliding strips -> [NCS, 16, 128, CW+3]
        strips = np.stack([xTc[:, i * CW : i * CW + CW + 3] for i in range(NCS)])
        strips = strips.reshape(NCS, KT, 128, CW + 3)
        # -> [NCS, 4 quarters, 128, 4*(CW+3)]
        quarters = np.ascontiguousarray(
            strips.reshape(NCS, KT // JQ, JQ, 128, CW + 3).transpose(0, 1, 3, 2, 4)
        ).reshape(NCS * (KT // JQ), 128, JQ * (CW + 3))
        xs0_np = np.ascontiguousarray(xTc[0:128, 0:260])
        xs0b_np = np.ascontiguousarray(xTc[0:128, 256:520])
        in_maps.append(
            {
                "xs_t": quarters,
                "xs0": xs0_np,
                "xs0b": xs0b_np,
                "wt": wt_np,
                "cw": cw_np,
                "cb": cb_np,
            }
        )
    return in_maps


def run_sharded(x, conv_w, conv_b, W_proj, b_proj, trace=False):
    """Run across the 8 cores; returns (full_out [B,S,D], BassKernelResults)."""
    from concourse.bass_utils import run_bass_kernel_spmd

    nc = _build_program()
    in_maps = _shard_inputs(x, conv_w, conv_b, W_proj, b_proj)
    try:
        res = run_bass_kernel_spmd(nc, in_maps, list(range(NCORES)), trace=trace)
    except Exception:
        # transient device wedges (NRT_EXEC_UNIT_UNRECOVERABLE) clear on retry
        res = run_bass_kernel_spmd(nc, in_maps, list(range(NCORES)), trace=trace)
    full = np.empty((B, S, D), dtype=np.float32)
    per_batch = S // T
    bp = b_proj.astype(np.float32)
    for c in range(NCORES):
        b = c // per_batch
        s0 = (c % per_batch) * T
        full[b, s0 : s0 + T] = res.results[c]["out"].astype(np.float32) + bp
    return full, res


def kernel(x, conv_w, conv_b, W_proj, b_proj):
    full, _ = run_sharded(x, conv_w, conv_b, W_proj, b_proj, trace=False)
    return full


# revision 53
# speedup vs baseline: 1.0260x; 1.0100x over previous
"""Trainium2 Bass kernel for nn_Expert_13082470383822.

y = silu(depthwise_causal_conv1d(x, conv_w, K=4) + conv_b);  out = y @ W_proj.T + b_proj
x [4, 4096, 2048] fp32. Data-parallel over the 16384 (batch*seq) tokens across
8 NeuronCores (2048 tokens/core + 3-token halo).

bf16 datapath. Per-core schedule:
- 72 dummy N=64 matmuls at kernel start keep the PE busy through the HAM
  activity window so the real matmuls run at 2.4 GHz from the first issue.
- strips 0-1 run the projection as j-outer phases of 8 PSUM banks covering
  ALL 4 m-tiles x 2 e-chunks of a 512-token strip; each W row-tile is split
  into column halves (wa = features 0-1023, wb = 1024-2047) so phase A only
  streams wa (half the W bandwidth demand of a full-tile phase) while wb
  arrives for phase B. Strips 2-3 run e-outer per m-tile (their conv is a
  full strip ahead by then) so copybacks overlap the next accumulation.
- each phase processes j14/j15 per bank-PAIR at staggered times so the 8
  PSUM copybacks (split 4 ACT / 4 DVE) pipeline with the next phase's
  restart instead of serializing against it.
- conv per j (full width): tap0 via DVE tensor_scalar, taps 2,3 via
  scalar_tensor_tensor, tap1 on ACT (Copy with per-partition scale), one
  DVE tensor_tensor combine, SiLU+conv_b on ACT. Strip-0 j0 is computed in
  halves from two tiny head tiles so the first matmul issues early.
- DMA: the startup-critical tiles (j0 head tiles + first x0 quarters) ride
  the ACT HWDGE queue, whose triggers fire earliest (only 4 — more blocks
  ACT compute on the ring); W halves + x3 + output stores on the sync HWDGE
  queue in first-use order; consts + the rest of x0, x1, x2 on the SWDGE
  gpsimd queue. b_proj is added on the host.
"""

import sys

if "/opt/trn_rl_repo" not in sys.path:
    sys.path.insert(0, "/opt/trn_rl_repo")

import os

import numpy as np

if os.environ.get("BASS_LDW_OPT", "0") == "1":
    import concourse.bass_utils as _bu

    if not getattr(_bu, "_ldw_opt_patched", False):
        _orig_run_command = _bu.run_command

        def _run_command_ldw(cmd, *a, **kw):
            cmd = [
                "--enable-ldw-opt=true" if c == "--enable-ldw-opt=false" else c
                for c in cmd
            ]
            return _orig_run_command(cmd, *a, **kw)

        _bu.run_command = _run_command_ldw
        _bu._ldw_opt_patched = True

B, S, D, KW = 4, 4096, 2048, 4
NCORES = 8
T = (B * S) // NCORES  # tokens per core = 2048
KT = D // 128  # 16 channel tiles
ECH = D // 512  # 4 e-chunks of the output features
CW = 512  # conv strip width (tokens)
MS = 128  # matmul stationary strip width (tokens)
NCS = T // CW  # 4 conv strips
MPC = CW // MS  # 4 matmul strips per conv strip
JQ = 4  # j-tiles per x quarter-load
NDUM = 72  # warm-up matmuls

_BUILT = {}


def _build_program():
    if "nc" in _BUILT:
        return _BUILT["nc"]

    import concourse.tile as tile
    from concourse import bacc, mybir

    dt = mybir.dt
    AF = mybir.ActivationFunctionType
    ALU = mybir.AluOpType

    nc = bacc.Bacc("TRN2", target_bir_lowering=False, debug=False)
    # pre-tiled x: per (conv-strip, j-quarter): [128, 4*(CW+3)] bf16 contiguous
    xs_d = nc.declare_dram_parameter(
        "xs_t", [NCS * (KT // JQ), 128, JQ * (CW + 3)], dt.bfloat16, isOutput=False
    )
    wt = nc.declare_dram_parameter("wt", [D, D], dt.bfloat16, isOutput=False)
    cw = nc.declare_dram_parameter("cw", [128, KT * KW], dt.float32, isOutput=False)
    cb = nc.declare_dram_parameter("cb", [128, KT], dt.float32, isOutput=False)
    out = nc.declare_dram_parameter("out", [T, D], dt.bfloat16, isOutput=True)

    with tile.TileContext(nc) as tc:
        with (
            tc.tile_pool(name="consts", bufs=1) as cpool,
            tc.tile_pool(name="wpool", bufs=1) as wpool,
            tc.tile_pool(name="xpool", bufs=12) as xpool,
            tc.tile_pool(name="ypool", bufs=3) as ypool,
            tc.tile_pool(name="apool", bufs=6) as apool,
            tc.tile_pool(name="opool", bufs=12) as opool,
            tc.tile_pool(name="pspool", bufs=8, space="PSUM") as pspool,
        ):
            xq = {}

            def load_xq(c, q, eng):
                xt = xpool.tile([128, JQ, CW + 3], dt.bfloat16, name="xs", tag="xs")
                eng.dma_start(
                    out=xt[:, :, :],
                    in_=xs_d[c * (KT // JQ) + q, :, :].rearrange(
                        "p (j t) -> p j t", j=JQ
                    ),
                )
                xq[(c, q)] = xt

            # ---- startup-critical x tiles ride the ACT HWDGE queue: its
            # triggers fire before the sync queue's preamble finishes; few
            # enough that the ring never blocks ACT compute. x0 quarter 0 is
            # split into two half-tiles: it paces strip-0's conv (and hence
            # all of phase A), and the first half lands ~3us sooner ----
            xq0h = []
            for h in range(2):
                xt = xpool.tile([128, 2, CW + 3], dt.bfloat16, name="xs", tag="xs")
                nc.scalar.dma_start(
                    out=xt[:, :, :],
                    in_=xs_d[0, :, h * 2 * (CW + 3) : (h + 1) * 2 * (CW + 3)].rearrange(
                        "p (j t) -> p j t", j=2
                    ),
                )
                xq0h.append(xt)
            load_xq(0, 1, nc.scalar)

            # ---- warm-up: ACT table + HAM via dummy matmuls ----
            dmw = cpool.tile([128, 64], dt.bfloat16, name="dmw")
            nc.vector.memset(dmw[:, :], 0.0)
            dum = cpool.tile([1, 1], dt.float32, name="dum")
            nc.vector.memset(dum[:, :], 0.0)
            nc.scalar.activation(dum[:, :], dum[:, :], AF.Silu, bias=0.0)
            dps = pspool.tile([128, 512], dt.float32, name="ps", tag="ps")
            for i in range(NDUM):
                nc.tensor.matmul(
                    dps[0:64, 0:64],
                    dmw[:, 0:64],
                    dmw[:, 0:64],
                    start=(i == 0),
                    stop=(i == NDUM - 1),
                )

            # ---- consts + the rest of x0, x1, x2 on the SWDGE gpsimd queue ----
            cw_sb = cpool.tile([128, KT * KW], dt.float32, name="cw_sb")
            nc.gpsimd.dma_start(out=cw_sb[:, :], in_=cw[:, :])
            cb_sb = cpool.tile([128, KT], dt.float32, name="cb_sb")
            nc.gpsimd.dma_start(out=cb_sb[:, :], in_=cb[:, :])
            load_xq(0, 2, nc.gpsimd)
            load_xq(0, 3, nc.gpsimd)
            for q in range(4):
                load_xq(1, q, nc.gpsimd)
            for q in range(4):
                load_xq(2, q, nc.gpsimd)

            # ---- W column-halves on sync in first-use order, then x3 ----
            wa = [None] * KT
            wb = [None] * KT
            for j in range(0, KT):
                wj = wpool.tile([128, 1024], dt.bfloat16, name=f"wa{j}")
                nc.sync.dma_start(
                    out=wj[:, :], in_=wt[j * 128 : (j + 1) * 128, 0:1024]
                )
                wa[j] = wj
            for j in range(KT):
                wj = wpool.tile([128, 1024], dt.bfloat16, name=f"wb{j}")
                nc.sync.dma_start(
                    out=wj[:, :], in_=wt[j * 128 : (j + 1) * 128, 1024:2048]
                )
                wb[j] = wj
            for q in range(4):
                load_xq(3, q, nc.sync)

            def wsel(j, e):
                w = wa[j] if e < 2 else wb[j]
                return w[:, (e % 2) * 512 : (e % 2) * 512 + 512]

            # ---- conv emission ----
            ys_strip = {}

            def conv_j(c, j):
                if c == 0 and j < 4:
                    xs = xq0h[j // 2]
                    jj = j % 2
                else:
                    xs = xq[(c, j // JQ)]
                    jj = j % JQ
                ys = ys_strip[c]
                t2 = apool.tile([128, 512], dt.bfloat16, name="t2", tag="t2")
                nc.scalar.activation(
                    t2[:, :], xs[:, jj, 1 : 1 + CW], AF.Copy,
                    scale=cw_sb[:, j * KW + 1 : j * KW + 2],
                )
                t1 = apool.tile([128, 512], dt.bfloat16, name="t1", tag="t1")
                nc.vector.tensor_scalar(
                    t1[:, :], xs[:, jj, 0:CW], cw_sb[:, j * KW : j * KW + 1],
                    None, ALU.mult,
                )
                for k in (2, 3):
                    nc.vector.scalar_tensor_tensor(
                        t1[:, :], xs[:, jj, k : k + CW],
                        cw_sb[:, j * KW + k : j * KW + k + 1],
                        t1[:, :], ALU.mult, ALU.add,
                    )
                t3 = apool.tile([128, 512], dt.bfloat16, name="t3", tag="t3")
                nc.vector.tensor_tensor(t3[:, :], t1[:, :], t2[:, :], ALU.add)
                nc.scalar.activation(
                    ys[:, j, :], t3[:, :], AF.Silu, bias=cb_sb[:, j : j + 1]
                )

            # ---- matmul phases: all 4 m-tiles x one e-half, j-outer ----
            def mm_phase(c, eh):
                ys = ys_strip[c]
                pss = [
                    pspool.tile([128, 512], dt.float32, name="ps", tag="ps")
                    for _ in range(8)
                ]

                def mm(b, j):
                    m, el = b // 2, b % 2
                    nc.tensor.matmul(
                        pss[b][:, :],
                        ys[:, j, m * MS : (m + 1) * MS],
                        wsel(j, 2 * eh + el),
                        start=(j == 0),
                        stop=(j == KT - 1),
                    )

                # j0..13 round-robin; last two j per bank-PAIR staggered so
                # copybacks pipeline with the next phase's restarts
                for j in range(KT - 2):
                    for b in range(8):
                        mm(b, j)
                for bp in range(4):
                    for j in (KT - 2, KT - 1):
                        for b in (2 * bp, 2 * bp + 1):
                            mm(b, j)
                return pss

            def copyback(c, eh, pss):
                for b in range(8):
                    m, el = b // 2, b % 2
                    e = 2 * eh + el
                    s = c * MPC + m
                    os_sb = opool.tile([128, 512], dt.bfloat16, name="os", tag="os")
                    if b % 2 == 0:
                        nc.scalar.copy(os_sb[:, :], pss[b][:, :])
                    else:
                        nc.vector.tensor_copy(os_sb[:, :], pss[b][:, :])
                    nc.sync.dma_start(
                        out=out[s * MS : (s + 1) * MS, e * 512 : (e + 1) * 512],
                        in_=os_sb[:, :],
                    )

            # ================= schedule =================
            ys_strip[0] = ypool.tile([128, KT, CW], dt.bfloat16, name="ys", tag="ys")
            for j in range(KT):
                conv_j(0, j)

            pss_a = mm_phase(0, 0)

            ys_strip[1] = ypool.tile([128, KT, CW], dt.bfloat16, name="ys", tag="ys")
            conv_j(1, 0)
            copyback(0, 0, pss_a)

            pss_b = mm_phase(0, 1)

            for j in range(1, 12):
                conv_j(1, j)
            copyback(0, 1, pss_b)
            for j in range(12, KT):
                conv_j(1, j)

            pss_c = mm_phase(1, 0)

            ys_strip[2] = ypool.tile([128, KT, CW], dt.bfloat16, name="ys", tag="ys")
            for j in range(0, 8):
                conv_j(2, j)
            copyback(1, 0, pss_c)

            pss_d = mm_phase(1, 1)

            for j in range(8, KT):
                conv_j(2, j)
            ys_strip[3] = ypool.tile([128, KT, CW], dt.bfloat16, name="ys", tag="ys")
            for j in range(0, 5):
                conv_j(3, j)
            copyback(1, 1, pss_d)

            # strips 2+3: conv is a full strip ahead by now, so e-outer per m
            # (copybacks overlap the next accumulation, no phase boundaries);
            # strip-3 conv interleaves with strip-2's m-tiles
            s3_conv = iter(range(5, KT))

            def emit_strip_eouter(c, conv_after_m=0):
                ys = ys_strip[c]
                for m in range(MPC):
                    s = c * MPC + m
                    pss = []
                    for e in range(ECH):
                        ps = pspool.tile([128, 512], dt.float32, name="ps", tag="ps")
                        for j in range(KT):
                            nc.tensor.matmul(
                                ps[:, :],
                                ys[:, j, m * MS : (m + 1) * MS],
                                wsel(j, e),
                                start=(j == 0),
                                stop=(j == KT - 1),
                            )
                        pss.append(ps)
                    for _ in range(conv_after_m):
                        j3 = next(s3_conv, None)
                        if j3 is not None:
                            conv_j(3, j3)
                    last = c == NCS - 1 and m == MPC - 1
                    for e in range(ECH):
                        os_sb = opool.tile(
                            [128, 512], dt.bfloat16, name="os", tag="os"
                        )
                        if last and e >= ECH - 2:
                            # final tiles: split the copy across ACT+DVE to
                            # shave the drain tail
                            nc.scalar.copy(os_sb[:, 0:256], pss[e][:, 0:256])
                            nc.vector.tensor_copy(
                                os_sb[:, 256:512], pss[e][:, 256:512]
                            )
                        elif e % 2 == 0:
                            nc.scalar.copy(os_sb[:, :], pss[e][:, :])
                        else:
                            nc.vector.tensor_copy(os_sb[:, :], pss[e][:, :])
                        nc.sync.dma_start(
                            out=out[s * MS : (s + 1) * MS, e * 512 : (e + 1) * 512],
                            in_=os_sb[:, :],
                        )

            emit_strip_eouter(2, conv_after_m=3)
            emit_strip_eouter(3)

    nc.compile()
    _BUILT["nc"] = nc
    return nc


def _shard_inputs(x, conv_w, conv_b, W_proj, b_proj):
    import ml_dtypes

    bf16 = ml_dtypes.bfloat16
    wt_np = np.ascontiguousarray(W_proj.T.astype(bf16))
    cw_np = np.ascontiguousarray(
        conv_w.reshape(KT, 128, KW).transpose(1, 0, 2).reshape(128, KT * KW),
        dtype=np.float32,
    )
    cb_np = np.ascontiguousarray(conv_b.reshape(KT, 128).T, dtype=np.float32)

    x16 = x.astype(bf16)
    per_batch = S // T
    in_maps = []
    for c in range(NCORES):
        b = c // per_batch
        s0 = (c % per_batch) * T
        xp = np.zeros((T + 3, D), dtype=bf16)
        xp[3:] = x16[b, s0 : s0 + T]
        if s0 > 0:
            xp[:3] = x16[b, s0 - 3 : s0]
        xTc = xp.T  # [D, T+3]
        # [NCS, D, CW+3] sliding strips -> [NCS, 16, 128, CW+3]
        strips = np.stack([xTc[:, i * CW : i * CW + CW + 3] for i in range(NCS)])
        strips = strips.reshape(NCS, KT, 128, CW + 3)
        # -> [NCS, 4 quarters, 128, 4*(CW+3)]
        quarters = np.ascontiguousarray(
            strips.reshape(NCS, KT // JQ, JQ, 128, CW + 3).transpose(0, 1, 3, 2, 4)
        ).reshape(NCS * (KT // JQ), 128, JQ * (CW + 3))
        in_maps.append(
            {
                "xs_t": quarters,
                "wt": wt_np,
                "cw": cw_np,
                "cb": cb_np,
            }
        )
    return in_maps


def run_sharded(x, conv_w, conv_b, W_proj, b_proj, trace=False):
    """Run across the 8 cores; returns (full_out [B,S,D], BassKernelResults)."""
    from concourse.bass_utils import run_bass_kernel_spmd

    nc = _build_program()
    in_maps = _shard_inputs(x, conv_w, conv_b, W_proj, b_proj)
    try:
        res = run_bass_kernel_spmd(nc, in_maps, list(range(NCORES)), trace=trace)
    except Exception:
        # transient device wedges (NRT_EXEC_UNIT_UNRECOVERABLE) clear on retry
        res = run_bass_kernel_spmd(nc, in_maps, list(range(NCORES)), trace=trace)
    full = np.empty((B, S, D), dtype=np.float32)
    per_batch = S // T
    bp = b_proj.astype(np.float32)
    for c in range(NCORES):
        b = c // per_batch
        s0 = (c % per_batch) * T
        full[b, s0 : s0 + T] = res.results[c]["out"].astype(np.float32) + bp
    return full, res


def kernel(x, conv_w, conv_b, W_proj, b_proj):
    full, _ = run_sharded(x, conv_w, conv_b, W_proj, b_proj, trace=False)
    return full
